# revision 4
# baseline (speedup 1.0000x reference)
"""Trainium2 Bass kernel for batched cross-attention.

Reference computation (fp32):
    scale = exp(min(logit_scale, log(100)))            # [P, 1]
    dots  = einsum("bpd,bnd->bpn", q, k) * scale       # [B, P, N]
    attn  = softmax(dots, axis=-1)
    out   = einsum("bpn,bnd->bpd", attn, v)            # [B, P, D]

Shapes: B=64, P=8, N=8192, D=256, fp32. k and v dominate traffic
(512 MB each) -> memory-bound. Sharding: data-parallel over B across the
8 NeuronCores (8 batches per core, ~128 MB of k+v per core).

Per-core kernel:
  - q is loaded in [p, b, d] layout, scaled by exp(min(logit_scale, c))
    (per-partition scalar), cast to fp16, and PE-transposed into
    qT [d, (b p)] tiles.
  - k streams in 1 MB chunks, cast fp32->fp16 during the SWDGE DMA.
    Each [128, 128] block is PE-transposed (identity matmul) into
    kT [d, n] tiles; QK^T runs as fp16 matmuls (qT stationary, kT
    moving, N=512) accumulating fp32 scores in PSUM. Four batches share
    one PSUM tile via PE column tiling (outputs at partition bases
    0/32/64/96), so scores for (batch g, part p) live on partition
    32 g + p of one of two fp32 scores [128, 8192] SBUF tiles.
  - Softmax in fp32 along the free dim: reduce_max (negated), one Exp
    activation with per-partition bias and fused free-dim sum
    (accum_out), reciprocal of the sum.
  - attn rows are PE-transposed into attnT [n, cols] fp16 tiles.
  - v streams in 1 MB chunks (cast fp16); attn@V accumulates into a
    PSUM [8, 256] tile per batch over all 64 n-tiles (attnT stationary,
    v moving); the PSUM->SBUF copy applies 1/sum per partition.

fp16 matmul inputs with fp32 accumulation + fp32 softmax give
rel-l2 ~2e-3 vs the fp32 reference (validated in numpy); bf16 would be
~1e-2 because the x10 logit scale amplifies mantissa error in the
softmax.
"""

import os
import sys
from contextlib import ExitStack

import numpy as np

for _p in ("/opt/trn_rl_repo", "/root/.axon_site/_ro/trn_rl_repo"):
    if os.path.isdir(_p) and _p not in sys.path:
        sys.path.insert(0, _p)

import concourse.bacc as bacc
import concourse.bass as bass  # noqa: F401
import concourse.tile as tile
from concourse import mybir
from concourse.bass_utils import run_bass_kernel_spmd
from concourse.masks import make_identity

AF = mybir.ActivationFunctionType
FP32 = mybir.dt.float32
FP16 = mybir.dt.float16

B, P, N, D = 64, 8, 8192, 256
NCORES = 8
BLOC = B // NCORES  # batches per core
LOG_SCALE_MAX = 4.6052  # log(100)

NCHUNK = 1024  # keys per DMA chunk (1 MB fp32 source)
NJ = N // NCHUNK  # chunks per batch
NSUB = NCHUNK // 128  # 128-row subtiles per chunk
NT = N // 128  # total n-tiles per batch
NG = 4  # batches per scores tile (PE column groups at 0/32/64/96)


def build():
    nc = bacc.Bacc("TRN2", target_bir_lowering=False, debug=False)
    q_d = nc.dram_tensor("q", [BLOC * P, D], FP32, kind="ExternalInput").ap()
    k_d = nc.dram_tensor("k", [BLOC, N, D], FP32, kind="ExternalInput").ap()
    v_d = nc.dram_tensor("v", [BLOC, N, D], FP32, kind="ExternalInput").ap()
    ls_d = nc.dram_tensor("logit_scale", [P, 1], FP32, kind="ExternalInput").ap()
    o_d = nc.dram_tensor("out", [BLOC * P, D], FP32, kind="ExternalOutput").ap()

    with tile.TileContext(nc) as tc, ExitStack() as ctx:
        singles = ctx.enter_context(tc.tile_pool(name="singles", bufs=1))
        loads = ctx.enter_context(tc.tile_pool(name="loads", bufs=3))
        kts = ctx.enter_context(tc.tile_pool(name="kts", bufs=3))
        outp = ctx.enter_context(tc.tile_pool(name="outp", bufs=2))

        ident16 = singles.tile([128, 128], FP16)
        make_identity(nc, ident16)
        ident32 = singles.tile([128, 128], FP32)
        make_identity(nc, ident32)

        # scale = exp(min(logit_scale, log 100)) on partitions 0..7
        s8 = singles.tile([P, 1], FP32)
        nc.sync.dma_start(out=s8, in_=ls_d)
        nc.vector.tensor_scalar_min(out=s8, in0=s8, scalar1=LOG_SCALE_MAX)
        nc.scalar.activation(out=s8, in_=s8, func=AF.Exp)

        # q [p, b, d] fp32, scaled by s8, cast fp16
        q_pbd = singles.tile([P, BLOC, D], FP32)
        nc.sync.dma_start(out=q_pbd, in_=q_d.rearrange("(b p) d -> p b d", p=P))
        q16 = singles.tile([P, BLOC, D], FP16)
        nc.vector.tensor_scalar_mul(out=q16, in0=q_pbd, scalar1=s8)

        # qT [128(d), 2(dh), 64(b p)] fp16
        qT = singles.tile([128, 2, BLOC * P], FP16)
        with tc.tile_pool(name="ps_setup", bufs=2, space="PSUM") as ps_setup:
            for b in range(BLOC):
                for dh in range(2):
                    ps = ps_setup.tile([128, P], FP16, tag="qt")
                    nc.tensor.transpose(
                        ps, q16[:, b, dh * 128 : (dh + 1) * 128], ident16[:P, :P]
                    )
                    nc.vector.tensor_copy(out=qT[:, dh, b * P : (b + 1) * P], in_=ps)

        # scores for (batch g2*4+g, part p) on partition 32g+p of scores_t[g2]
        scores_t = [
            singles.tile([128, N], FP32, tag=f"scores{g2}", name=f"scores{g2}")
            for g2 in range(2)
        ]

        # ---------------- k phase: scores = (q*s) @ k^T ----------------
        with (
            tc.tile_pool(name="ps_kt", bufs=2, space="PSUM") as ps_kt,
            tc.tile_pool(name="ps_sc", bufs=2, space="PSUM") as ps_sc,
        ):
            for j in range(NJ):
                for g2 in range(2):
                    sc_ps = ps_sc.tile([128, NCHUNK], FP32, tag="sc")
                    if j == 0:
                        # first use of each slot: clear stale garbage on the
                        # partition rows no matmul writes (8..31 etc.)
                        nc.vector.memset(sc_ps, 0.0)
                    for g in range(NG):
                        b = g2 * NG + g
                        knat = loads.tile([128, NSUB, D], FP16, tag="knat")
                        nc.gpsimd.dma_start(
                            out=knat,
                            in_=k_d[b, j * NCHUNK : (j + 1) * NCHUNK, :].rearrange(
                                "(s p) d -> p s d", p=128
                            ),
                        )
                        for dh in range(2):
                            kt_ps = ps_kt.tile([128, NCHUNK], FP16, tag="kt")
                            for s in range(NSUB):
                                nc.tensor.transpose(
                                    kt_ps[:, s * 128 : (s + 1) * 128],
                                    knat[:, s, dh * 128 : (dh + 1) * 128],
                                    ident16,
                                )
                            kt_sb = kts.tile([128, NCHUNK], FP16, tag="kt_sb")
                            nc.vector.tensor_copy(out=kt_sb, in_=kt_ps)
                            for h in range(NCHUNK // 512):
                                nc.tensor.matmul(
                                    sc_ps[
                                        32 * g : 32 * g + P, h * 512 : (h + 1) * 512
                                    ],
                                    qT[:, dh, b * P : (b + 1) * P],
                                    kt_sb[:, h * 512 : (h + 1) * 512],
                                    start=(dh == 0),
                                    stop=(dh == 1),
                                    tile_position=(0, 32 * g),
                                )
                    nc.scalar.copy(
                        out=scores_t[g2][:, j * NCHUNK : (j + 1) * NCHUNK],
                        in_=sc_ps,
                    )

        # ---------------- softmax over n (free dim), fp32 ----------------
        rsum_pb = singles.tile([P, BLOC], FP32)  # 1/sum laid out [p, b]
        dram = ctx.enter_context(tc.tile_pool(name="dramtmp", bufs=1, space="DRAM"))
        for g2 in range(2):
            neg_max = singles.tile([128, 1], FP32, tag=f"negmax{g2}")
            nc.vector.reduce_max(
                out=neg_max,
                in_=scores_t[g2],
                axis=mybir.AxisListType.X,
                negate=True,
            )
            sumexp = singles.tile([128, 1], FP32, tag=f"sumexp{g2}")
            nc.scalar.activation(
                out=scores_t[g2],
                in_=scores_t[g2],
                func=AF.Exp,
                bias=neg_max,
                accum_out=sumexp,
            )
            rsum = singles.tile([128, 1], FP32, tag=f"rsum{g2}")
            nc.vector.reciprocal(out=rsum, in_=sumexp)
            rs_dram = dram.tile([128, 1], FP32, tag=f"rsd{g2}")
            nc.sync.dma_start(out=rs_dram, in_=rsum)
            # rsum_pb[p, g2*4+g] = rsum[32 g + p]
            src = bass.AP(
                tensor=rs_dram.tensor,
                offset=rs_dram.offset,
                ap=[[1, P], [32, NG]],
            )
            nc.sync.dma_start(out=rsum_pb[:, g2 * NG : (g2 + 1) * NG], in_=src)

        # attnT [128(n), NT, 128(cols)] fp16 per scores tile
        attnT = [
            singles.tile([128, NT, 128], FP16, tag=f"attnT{g2}", name=f"attnT{g2}")
            for g2 in range(2)
        ]
        with tc.tile_pool(name="ps_at", bufs=4, space="PSUM") as ps_at:
            for t in range(NT):
                for g2 in range(2):
                    at_ps = ps_at.tile([128, 128], FP32, tag="at")
                    nc.tensor.transpose(
                        at_ps, scores_t[g2][:, t * 128 : (t + 1) * 128], ident32
                    )
                    nc.vector.tensor_copy(out=attnT[g2][:, t, :], in_=at_ps)

        # ---------------- v phase: out = attn @ v ----------------
        with tc.tile_pool(name="ps_out", bufs=2, space="PSUM") as ps_out:
            for b in range(BLOC):
                g2, g = b // NG, b % NG
                o_ps = ps_out.tile([P, D], FP32, tag="o")
                for j in range(NJ):
                    vnat = loads.tile([128, NSUB, D], FP16, tag="vnat")
                    nc.gpsimd.dma_start(
                        out=vnat,
                        in_=v_d[b, j * NCHUNK : (j + 1) * NCHUNK, :].rearrange(
                            "(s p) d -> p s d", p=128
                        ),
                    )
                    for s in range(NSUB):
                        t = j * NSUB + s
                        nc.tensor.matmul(
                            o_ps,
                            attnT[g2][:, t, 32 * g : 32 * g + P],
                            vnat[:, s, :],
                            start=(t == 0),
                            stop=(t == NT - 1),
                        )
                o_sb = outp.tile([P, D], FP32, tag="o_sb")
                nc.vector.tensor_scalar_mul(
                    out=o_sb, in0=o_ps, scalar1=rsum_pb[:, b : b + 1]
                )
                nc.sync.dma_start(out=o_d[b * P : (b + 1) * P, :], in_=o_sb)

    nc.compile()
    return nc


_NC_CACHE = None


def _get_nc():
    global _NC_CACHE
    if _NC_CACHE is None:
        _NC_CACHE = build()
    return _NC_CACHE


def make_in_maps(q, k, v, ls):
    in_maps = []
    for c in range(NCORES):
        sl = slice(c * BLOC, (c + 1) * BLOC)
        in_maps.append(
            {
                "q": np.ascontiguousarray(q[sl].reshape(BLOC * P, D)),
                "k": np.ascontiguousarray(k[sl]),
                "v": np.ascontiguousarray(v[sl]),
                "logit_scale": np.ascontiguousarray(ls),
            }
        )
    return in_maps


def kernel(**inputs) -> np.ndarray:
    q = np.asarray(inputs["q"], dtype=np.float32)
    k = np.asarray(inputs["k"], dtype=np.float32)
    v = np.asarray(inputs["v"], dtype=np.float32)
    ls = np.asarray(inputs["logit_scale"], dtype=np.float32).reshape(P, 1)

    nc = _get_nc()
    in_maps = make_in_maps(q, k, v, ls)
    res = run_bass_kernel_spmd(nc, in_maps, core_ids=list(range(NCORES)))
    out = np.concatenate(
        [r["out"].reshape(BLOC, P, D) for r in res.results], axis=0
    )
    return out


if __name__ == "__main__":
    rng = np.random.default_rng(0)
    inputs = {
        "q": rng.standard_normal((B, P, D), dtype=np.float32),
        "k": rng.standard_normal((B, N, D), dtype=np.float32),
        "v": rng.standard_normal((B, N, D), dtype=np.float32),
        "logit_scale": np.log(10.0 * np.ones((P, 1), dtype=np.float32)),
    }
    out = kernel(**inputs)
    print("out", out.shape, out.dtype)


# revision 6
# speedup vs baseline: 82.2303x; 82.2303x over previous
"""Trainium2 Bass kernel for batched cross-attention.

Reference computation (fp32):
    scale = exp(min(logit_scale, log(100)))            # [P, 1]
    dots  = einsum("bpd,bnd->bpn", q, k) * scale       # [B, P, N]
    attn  = softmax(dots, axis=-1)
    out   = einsum("bpn,bnd->bpd", attn, v)            # [B, P, D]

Shapes: B=64, P=8, N=8192, D=256, fp32. k and v dominate traffic
(512 MB each) -> memory-bound. Sharding: data-parallel over B across the
8 NeuronCores (8 batches per core, ~128 MB of k+v per core).

Per-core kernel:
  - q is loaded in [p, b, d] layout, scaled by exp(min(logit_scale, c))
    (per-partition scalar), cast to fp16, and PE-transposed into
    qT [d, (b p)] tiles.
  - k streams in 1 MB fp32 chunks.
    Each
    [128, 128] block is PE-transposed (identity matmul) into kT [d, n]
    tiles; QK^T runs as fp32 matmuls (qT stationary, kT moving, N=512)
    accumulating fp32 scores in PSUM. Four batches share
    one PSUM tile via PE column tiling (outputs at partition bases
    0/32/64/96), so scores for (batch g, part p) live on partition
    32 g + p of one of two fp32 scores [128, 8192] SBUF tiles.
  - Softmax in fp32 along the free dim: reduce_max (negated), one Exp
    activation with per-partition bias and fused free-dim sum
    (accum_out), reciprocal of the sum.
  - attn rows are PE-transposed into attnT [n, cols] fp16 tiles.
  - v streams in 1 MB chunks (cast fp16); attn@V accumulates into a
    PSUM [8, 256] tile per batch over all 64 n-tiles (attnT stationary,
    v moving); the PSUM->SBUF copy applies 1/sum per partition.

QK^T runs in fp32 (exact logits; the x10 logit scale amplifies any
mantissa error through the softmax), attn@V in fp16 with fp32
accumulation: rel-l2 ~2e-4 vs the fp32 reference (validated in numpy).
"""

import os
import sys
from contextlib import ExitStack

import numpy as np

for _p in ("/opt/trn_rl_repo", "/root/.axon_site/_ro/trn_rl_repo"):
    if os.path.isdir(_p) and _p not in sys.path:
        sys.path.insert(0, _p)

import concourse.bacc as bacc
import concourse.bass as bass  # noqa: F401
import concourse.tile as tile
from concourse import mybir
from concourse.bass_utils import run_bass_kernel_spmd
from concourse.masks import make_identity

AF = mybir.ActivationFunctionType
FP32 = mybir.dt.float32
FP16 = mybir.dt.float16

B, P, N, D = 64, 8, 8192, 256
NCORES = 8
BLOC = B // NCORES  # batches per core
LOG_SCALE_MAX = 4.6052  # log(100)

NCHUNK = 1024  # keys per DMA chunk (1 MB fp32 source)
NJ = N // NCHUNK  # chunks per batch
NSUB = NCHUNK // 128  # 128-row subtiles per chunk
NT = N // 128  # total n-tiles per batch
NG = 4  # batches per scores tile (PE column groups at 0/32/64/96)


def build(reps=None):
    """Build the kernel; with reps, wrap the body in a HW For loop (timing)."""
    nc = bacc.Bacc("TRN2", target_bir_lowering=False, debug=False)
    q_d = nc.dram_tensor("q", [BLOC * P, D], FP32, kind="ExternalInput").ap()
    k_d = nc.dram_tensor("k", [BLOC, N, D], FP32, kind="ExternalInput").ap()
    v_d = nc.dram_tensor("v", [BLOC, N, D], FP32, kind="ExternalInput").ap()
    ls_d = nc.dram_tensor("logit_scale", [P, 1], FP32, kind="ExternalInput").ap()
    o_d = nc.dram_tensor("out", [BLOC * P, D], FP32, kind="ExternalOutput").ap()

    with tile.TileContext(nc) as tc, ExitStack() as ctx:
        if reps is not None:
            ctx.enter_context(tc.For_i(0, reps, 1))
        singles = ctx.enter_context(tc.tile_pool(name="singles", bufs=1))
        loads = ctx.enter_context(tc.tile_pool(name="loads", bufs=3))
        kts = ctx.enter_context(tc.tile_pool(name="kts", bufs=3))
        outp = ctx.enter_context(tc.tile_pool(name="outp", bufs=2))

        ident32 = singles.tile([128, 128], FP32)
        make_identity(nc, ident32)

        # scale = exp(min(logit_scale, log 100)) on partitions 0..7
        s8 = singles.tile([P, 1], FP32)
        nc.sync.dma_start(out=s8, in_=ls_d)
        nc.vector.tensor_scalar_min(out=s8, in0=s8, scalar1=LOG_SCALE_MAX)
        nc.scalar.activation(out=s8, in_=s8, func=AF.Exp)

        # q [p, b, d] fp32, scaled by s8
        q_pbd = singles.tile([P, BLOC, D], FP32)
        nc.sync.dma_start(out=q_pbd, in_=q_d.rearrange("(b p) d -> p b d", p=P))
        qs = singles.tile([P, BLOC, D], FP32)
        nc.vector.tensor_scalar_mul(out=qs, in0=q_pbd, scalar1=s8)

        # qT [128(d), 2(dh), 64(b p)] fp32
        qT = singles.tile([128, 2, BLOC * P], FP32)
        with tc.tile_pool(name="ps_setup", bufs=2, space="PSUM") as ps_setup:
            for b in range(BLOC):
                for dh in range(2):
                    ps = ps_setup.tile([128, P], FP32, tag="qt")
                    nc.tensor.transpose(
                        ps, qs[:, b, dh * 128 : (dh + 1) * 128], ident32[:P, :P]
                    )
                    nc.vector.tensor_copy(out=qT[:, dh, b * P : (b + 1) * P], in_=ps)

        # scores for (batch g2*4+g, part p) on partition 32g+p of scores_t[g2]
        scores_t = [
            singles.tile([128, N], FP32, tag=f"scores{g2}", name=f"scores{g2}")
            for g2 in range(2)
        ]

        # ---------------- k phase: scores = (q*s) @ k^T ----------------
        with (
            tc.tile_pool(name="ps_kt", bufs=2, space="PSUM") as ps_kt,
            tc.tile_pool(name="ps_sc", bufs=2, space="PSUM") as ps_sc,
        ):
            for j in range(NJ):
                for g2 in range(2):
                    sc_ps = ps_sc.tile([128, NCHUNK], FP32, tag="sc")
                    if j == 0:
                        # first use of each slot: clear stale garbage on the
                        # partition rows no matmul writes (8..31 etc.)
                        nc.vector.memset(sc_ps, 0.0)
                    for g in range(NG):
                        b = g2 * NG + g
                        knat = loads.tile([128, NSUB, D], FP32, tag="knat")
                        nc.sync.dma_start(
                            out=knat,
                            in_=k_d[b, j * NCHUNK : (j + 1) * NCHUNK, :].rearrange(
                                "(s p) d -> p s d", p=128
                            ),
                        )
                        for dh in range(2):
                            for h in range(2):
                                kt_ps = ps_kt.tile([128, 512], FP32, tag="kt")
                                for s in range(4):
                                    nc.tensor.transpose(
                                        kt_ps[:, s * 128 : (s + 1) * 128],
                                        knat[
                                            :,
                                            h * 4 + s,
                                            dh * 128 : (dh + 1) * 128,
                                        ],
                                        ident32,
                                    )
                                kt_sb = kts.tile([128, 512], FP32, tag="kt_sb")
                                nc.vector.tensor_copy(out=kt_sb, in_=kt_ps)
                                nc.tensor.matmul(
                                    sc_ps[
                                        32 * g : 32 * g + P, h * 512 : (h + 1) * 512
                                    ],
                                    qT[:, dh, b * P : (b + 1) * P],
                                    kt_sb,
                                    start=(dh == 0),
                                    stop=(dh == 1),
                                    tile_position=(0, 32 * g),
                                )
                    nc.scalar.copy(
                        out=scores_t[g2][:, j * NCHUNK : (j + 1) * NCHUNK],
                        in_=sc_ps,
                    )

        # ---------------- softmax over n (free dim), fp32 ----------------
        rsum_pb = singles.tile([P, BLOC], FP32)  # 1/sum laid out [p, b]
        dram = ctx.enter_context(tc.tile_pool(name="dramtmp", bufs=1, space="DRAM"))
        for g2 in range(2):
            neg_max = singles.tile([128, 1], FP32, tag=f"negmax{g2}")
            nc.vector.reduce_max(
                out=neg_max,
                in_=scores_t[g2],
                axis=mybir.AxisListType.X,
                negate=True,
            )
            sumexp = singles.tile([128, 1], FP32, tag=f"sumexp{g2}")
            nc.scalar.activation(
                out=scores_t[g2],
                in_=scores_t[g2],
                func=AF.Exp,
                bias=neg_max,
                accum_out=sumexp,
            )
            rsum = singles.tile([128, 1], FP32, tag=f"rsum{g2}")
            nc.vector.reciprocal(out=rsum, in_=sumexp)
            rs_dram = dram.tile([128, 1], FP32, tag=f"rsd{g2}")
            nc.sync.dma_start(out=rs_dram, in_=rsum)
            # rsum_pb[p, g2*4+g] = rsum[32 g + p]
            src = bass.AP(
                tensor=rs_dram.tensor,
                offset=rs_dram.offset,
                ap=[[1, P], [32, NG]],
            )
            nc.sync.dma_start(out=rsum_pb[:, g2 * NG : (g2 + 1) * NG], in_=src)

        # attnT [128(n), NT, 128(cols)] fp16 per scores tile
        attnT = [
            singles.tile([128, NT, 128], FP16, tag=f"attnT{g2}", name=f"attnT{g2}")
            for g2 in range(2)
        ]
        with tc.tile_pool(name="ps_at", bufs=4, space="PSUM") as ps_at:
            for t in range(NT):
                for g2 in range(2):
                    at_ps = ps_at.tile([128, 128], FP32, tag="at")
                    nc.tensor.transpose(
                        at_ps, scores_t[g2][:, t * 128 : (t + 1) * 128], ident32
                    )
                    nc.vector.tensor_copy(out=attnT[g2][:, t, :], in_=at_ps)

        # ---------------- v phase: out = attn @ v ----------------
        with tc.tile_pool(name="ps_out", bufs=2, space="PSUM") as ps_out:
            for b in range(BLOC):
                g2, g = b // NG, b % NG
                o_ps = ps_out.tile([P, D], FP32, tag="o")
                for j in range(NJ):
                    vnat = loads.tile([128, NSUB, D], FP16, tag="vnat")
                    nc.gpsimd.dma_start(
                        out=vnat,
                        in_=v_d[b, j * NCHUNK : (j + 1) * NCHUNK, :].rearrange(
                            "(s p) d -> p s d", p=128
                        ),
                    )
                    for s in range(NSUB):
                        t = j * NSUB + s
                        nc.tensor.matmul(
                            o_ps,
                            attnT[g2][:, t, 32 * g : 32 * g + P],
                            vnat[:, s, :],
                            start=(t == 0),
                            stop=(t == NT - 1),
                        )
                o_sb = outp.tile([P, D], FP32, tag="o_sb")
                nc.vector.tensor_scalar_mul(
                    out=o_sb, in0=o_ps, scalar1=rsum_pb[:, b : b + 1]
                )
                nc.sync.dma_start(out=o_d[b * P : (b + 1) * P, :], in_=o_sb)

    nc.compile()
    return nc


_NC_CACHE = None


def _get_nc():
    global _NC_CACHE
    if _NC_CACHE is None:
        _NC_CACHE = build()
    return _NC_CACHE


def make_in_maps(q, k, v, ls):
    in_maps = []
    for c in range(NCORES):
        sl = slice(c * BLOC, (c + 1) * BLOC)
        in_maps.append(
            {
                "q": np.ascontiguousarray(q[sl].reshape(BLOC * P, D)),
                "k": np.ascontiguousarray(k[sl]),
                "v": np.ascontiguousarray(v[sl]),
                "logit_scale": np.ascontiguousarray(ls),
            }
        )
    return in_maps


def kernel(**inputs) -> np.ndarray:
    q = np.asarray(inputs["q"], dtype=np.float32)
    k = np.asarray(inputs["k"], dtype=np.float32)
    v = np.asarray(inputs["v"], dtype=np.float32)
    ls = np.asarray(inputs["logit_scale"], dtype=np.float32).reshape(P, 1)

    nc = _get_nc()
    in_maps = make_in_maps(q, k, v, ls)
    res = run_bass_kernel_spmd(nc, in_maps, core_ids=list(range(NCORES)))
    out = np.concatenate(
        [r["out"].reshape(BLOC, P, D) for r in res.results], axis=0
    )
    return out


if __name__ == "__main__":
    rng = np.random.default_rng(0)
    inputs = {
        "q": rng.standard_normal((B, P, D), dtype=np.float32),
        "k": rng.standard_normal((B, N, D), dtype=np.float32),
        "v": rng.standard_normal((B, N, D), dtype=np.float32),
        "logit_scale": np.log(10.0 * np.ones((P, 1), dtype=np.float32)),
    }
    out = kernel(**inputs)
    print("out", out.shape, out.dtype)


# revision 7
# speedup vs baseline: 2245.2519x; 27.3044x over previous
"""Trainium2 Bass kernel for batched cross-attention.

Reference computation (fp32):
    scale = exp(min(logit_scale, log(100)))            # [P, 1]
    dots  = einsum("bpd,bnd->bpn", q, k) * scale       # [B, P, N]
    attn  = softmax(dots, axis=-1)
    out   = einsum("bpn,bnd->bpd", attn, v)            # [B, P, D]

Shapes: B=64, P=8, N=8192, D=256, fp32. k and v dominate traffic
(512 MB each) -> memory-bound. Sharding: data-parallel over B across the
8 NeuronCores (8 batches per core, ~128 MB of k+v per core).

Per-core kernel:
  - q is loaded in [p, b, d] layout, scaled by exp(min(logit_scale, c))
    (per-partition scalar), cast to fp16, and PE-transposed into
    qT [d, (b p)] tiles.
  - k streams in 1 MB fp32 chunks.
    Each
    [128, 128] block is PE-transposed (identity matmul) into kT [d, n]
    tiles; QK^T runs as fp32 matmuls (qT stationary, kT moving, N=512)
    accumulating fp32 scores in PSUM. Four batches share
    one PSUM tile via PE column tiling (outputs at partition bases
    0/32/64/96), so scores for (batch g, part p) live on partition
    32 g + p of one of two fp32 scores [128, 8192] SBUF tiles.
  - Softmax in fp32 along the free dim: reduce_max (negated), one Exp
    activation with per-partition bias and fused free-dim sum
    (accum_out), reciprocal of the sum.
  - attn rows are PE-transposed into attnT [n, cols] fp16 tiles.
  - v streams in 1 MB chunks (cast fp16); attn@V accumulates into a
    PSUM [8, 256] tile per batch over all 64 n-tiles (attnT stationary,
    v moving); the PSUM->SBUF copy applies 1/sum per partition.

QK^T runs in fp32 (exact logits; the x10 logit scale amplifies any
mantissa error through the softmax), attn@V in fp16 with fp32
accumulation: rel-l2 ~2e-4 vs the fp32 reference (validated in numpy).
"""

import os
import sys
from contextlib import ExitStack

import numpy as np

for _p in ("/opt/trn_rl_repo", "/root/.axon_site/_ro/trn_rl_repo"):
    if os.path.isdir(_p) and _p not in sys.path:
        sys.path.insert(0, _p)

import concourse.bacc as bacc
import concourse.bass as bass  # noqa: F401
import concourse.tile as tile
from concourse import mybir
from concourse.bass_utils import run_bass_kernel_spmd
from concourse.masks import make_identity

AF = mybir.ActivationFunctionType
FP32 = mybir.dt.float32
FP16 = mybir.dt.float16

B, P, N, D = 64, 8, 8192, 256
NCORES = 8
BLOC = B // NCORES  # batches per core
LOG_SCALE_MAX = 4.6052  # log(100)

NCHUNK = 1024  # keys per DMA chunk (1 MB fp32 source)
NJ = N // NCHUNK  # chunks per batch
NSUB = NCHUNK // 128  # 128-row subtiles per chunk
NT = N // 128  # total n-tiles per batch
NG = 4  # batches per scores tile (PE column groups at 0/32/64/96)


def build(reps=1):
    """Build the kernel; reps>1 statically unrolls the whole body (timing)."""
    nc = bacc.Bacc("TRN2", target_bir_lowering=False, debug=False)
    q_d = nc.dram_tensor("q", [BLOC * P, D], FP32, kind="ExternalInput").ap()
    k_d = nc.dram_tensor("k", [BLOC, N, D], FP32, kind="ExternalInput").ap()
    v_d = nc.dram_tensor("v", [BLOC, N, D], FP32, kind="ExternalInput").ap()
    ls_d = nc.dram_tensor("logit_scale", [P, 1], FP32, kind="ExternalInput").ap()
    o_d = nc.dram_tensor("out", [BLOC * P, D], FP32, kind="ExternalOutput").ap()

    with tile.TileContext(nc) as tc, ExitStack() as ctx:
        singles = ctx.enter_context(tc.tile_pool(name="singles", bufs=1))
        loads = ctx.enter_context(tc.tile_pool(name="loads", bufs=3))
        kts = ctx.enter_context(tc.tile_pool(name="kts", bufs=3))
        outp = ctx.enter_context(tc.tile_pool(name="outp", bufs=2))

        ident32 = singles.tile([128, 128], FP32)
        make_identity(nc, ident32)

        for rep in range(reps):
            _emit_body(
                nc, tc, ctx, singles, loads, kts, outp, ident32,
                q_d, k_d, v_d, ls_d, o_d, rep,
            )

    nc.compile()
    return nc


def _emit_body(nc, tc, ctx, singles, loads, kts, outp, ident32,
               q_d, k_d, v_d, ls_d, o_d, rep):
    if True:
        # scale = exp(min(logit_scale, log 100)) on partitions 0..7
        s8 = singles.tile([P, 1], FP32)
        nc.sync.dma_start(out=s8, in_=ls_d)
        nc.vector.tensor_scalar_min(out=s8, in0=s8, scalar1=LOG_SCALE_MAX)
        nc.scalar.activation(out=s8, in_=s8, func=AF.Exp)

        # q [p, b, d] fp32, scaled by s8
        q_pbd = singles.tile([P, BLOC, D], FP32)
        nc.sync.dma_start(out=q_pbd, in_=q_d.rearrange("(b p) d -> p b d", p=P))
        qs = singles.tile([P, BLOC, D], FP32)
        nc.vector.tensor_scalar_mul(out=qs, in0=q_pbd, scalar1=s8)

        # qT [128(d), 2(dh), 64(b p)] fp32
        qT = singles.tile([128, 2, BLOC * P], FP32)
        with tc.tile_pool(name=f"ps_setup{rep}", bufs=2, space="PSUM") as ps_setup:
            for b in range(BLOC):
                for dh in range(2):
                    ps = ps_setup.tile([128, P], FP32, tag="qt")
                    nc.tensor.transpose(
                        ps, qs[:, b, dh * 128 : (dh + 1) * 128], ident32[:P, :P]
                    )
                    nc.vector.tensor_copy(out=qT[:, dh, b * P : (b + 1) * P], in_=ps)

        # scores for (batch g2*4+g, part p) on partition 32g+p of scores_t[g2]
        scores_t = [
            singles.tile([128, N], FP32, tag=f"scores{g2}", name=f"scores{g2}")
            for g2 in range(2)
        ]

        # ---------------- k phase: scores = (q*s) @ k^T ----------------
        with (
            tc.tile_pool(name=f"ps_kt{rep}", bufs=2, space="PSUM") as ps_kt,
            tc.tile_pool(name=f"ps_sc{rep}", bufs=2, space="PSUM") as ps_sc,
        ):
            for j in range(NJ):
                for g2 in range(2):
                    sc_ps = ps_sc.tile([128, NCHUNK], FP32, tag="sc")
                    if j == 0:
                        # first use of each slot: clear stale garbage on the
                        # partition rows no matmul writes (8..31 etc.)
                        nc.vector.memset(sc_ps, 0.0)
                    for g in range(NG):
                        b = g2 * NG + g
                        knat = loads.tile([128, NSUB, D], FP32, tag="knat")
                        nc.sync.dma_start(
                            out=knat,
                            in_=k_d[b, j * NCHUNK : (j + 1) * NCHUNK, :].rearrange(
                                "(s p) d -> p s d", p=128
                            ),
                        )
                        for dh in range(2):
                            for h in range(2):
                                kt_ps = ps_kt.tile([128, 512], FP32, tag="kt")
                                for s in range(4):
                                    nc.tensor.transpose(
                                        kt_ps[:, s * 128 : (s + 1) * 128],
                                        knat[
                                            :,
                                            h * 4 + s,
                                            dh * 128 : (dh + 1) * 128,
                                        ],
                                        ident32,
                                    )
                                kt_sb = kts.tile([128, 512], FP32, tag="kt_sb")
                                nc.vector.tensor_copy(out=kt_sb, in_=kt_ps)
                                nc.tensor.matmul(
                                    sc_ps[
                                        32 * g : 32 * g + P, h * 512 : (h + 1) * 512
                                    ],
                                    qT[:, dh, b * P : (b + 1) * P],
                                    kt_sb,
                                    start=(dh == 0),
                                    stop=(dh == 1),
                                    tile_position=(0, 32 * g),
                                )
                    nc.scalar.copy(
                        out=scores_t[g2][:, j * NCHUNK : (j + 1) * NCHUNK],
                        in_=sc_ps,
                    )

        # ---------------- softmax over n (free dim), fp32 ----------------
        rsum_pb = singles.tile([P, BLOC], FP32)  # 1/sum laid out [p, b]
        dram = ctx.enter_context(tc.tile_pool(name=f"dramtmp{rep}", bufs=1, space="DRAM"))
        for g2 in range(2):
            neg_max = singles.tile([128, 1], FP32, tag=f"negmax{g2}")
            nc.vector.reduce_max(
                out=neg_max,
                in_=scores_t[g2],
                axis=mybir.AxisListType.X,
                negate=True,
            )
            sumexp = singles.tile([128, 1], FP32, tag=f"sumexp{g2}")
            nc.scalar.activation(
                out=scores_t[g2],
                in_=scores_t[g2],
                func=AF.Exp,
                bias=neg_max,
                accum_out=sumexp,
            )
            rsum = singles.tile([128, 1], FP32, tag=f"rsum{g2}")
            nc.vector.reciprocal(out=rsum, in_=sumexp)
            rs_dram = dram.tile([128, 1], FP32, tag=f"rsd{g2}")
            nc.sync.dma_start(out=rs_dram, in_=rsum)
            # rsum_pb[p, g2*4+g] = rsum[32 g + p]
            src = bass.AP(
                tensor=rs_dram.tensor,
                offset=rs_dram.offset,
                ap=[[1, P], [32, NG]],
            )
            nc.sync.dma_start(out=rsum_pb[:, g2 * NG : (g2 + 1) * NG], in_=src)

        # attnT [128(n), NT, 128(cols)] fp16 per scores tile
        attnT = [
            singles.tile([128, NT, 128], FP16, tag=f"attnT{g2}", name=f"attnT{g2}")
            for g2 in range(2)
        ]
        with tc.tile_pool(name=f"ps_at{rep}", bufs=4, space="PSUM") as ps_at:
            for t in range(NT):
                for g2 in range(2):
                    at_ps = ps_at.tile([128, 128], FP32, tag="at")
                    nc.tensor.transpose(
                        at_ps, scores_t[g2][:, t * 128 : (t + 1) * 128], ident32
                    )
                    nc.vector.tensor_copy(out=attnT[g2][:, t, :], in_=at_ps)

        # ---------------- v phase: out = attn @ v ----------------
        with tc.tile_pool(name=f"ps_out{rep}", bufs=2, space="PSUM") as ps_out:
            for b in range(BLOC):
                g2, g = b // NG, b % NG
                o_ps = ps_out.tile([P, D], FP32, tag="o")
                for j in range(NJ):
                    vnat = loads.tile([128, NSUB, D], FP16, tag="vnat")
                    nc.gpsimd.dma_start(
                        out=vnat,
                        in_=v_d[b, j * NCHUNK : (j + 1) * NCHUNK, :].rearrange(
                            "(s p) d -> p s d", p=128
                        ),
                    )
                    for s in range(NSUB):
                        t = j * NSUB + s
                        nc.tensor.matmul(
                            o_ps,
                            attnT[g2][:, t, 32 * g : 32 * g + P],
                            vnat[:, s, :],
                            start=(t == 0),
                            stop=(t == NT - 1),
                        )
                o_sb = outp.tile([P, D], FP32, tag="o_sb")
                nc.vector.tensor_scalar_mul(
                    out=o_sb, in0=o_ps, scalar1=rsum_pb[:, b : b + 1]
                )
                nc.sync.dma_start(out=o_d[b * P : (b + 1) * P, :], in_=o_sb)


_NC_CACHE = None


def _get_nc():
    global _NC_CACHE
    if _NC_CACHE is None:
        _NC_CACHE = build()
    return _NC_CACHE


def make_in_maps(q, k, v, ls):
    in_maps = []
    for c in range(NCORES):
        sl = slice(c * BLOC, (c + 1) * BLOC)
        in_maps.append(
            {
                "q": np.ascontiguousarray(q[sl].reshape(BLOC * P, D)),
                "k": np.ascontiguousarray(k[sl]),
                "v": np.ascontiguousarray(v[sl]),
                "logit_scale": np.ascontiguousarray(ls),
            }
        )
    return in_maps


def kernel(**inputs) -> np.ndarray:
    q = np.asarray(inputs["q"], dtype=np.float32)
    k = np.asarray(inputs["k"], dtype=np.float32)
    v = np.asarray(inputs["v"], dtype=np.float32)
    ls = np.asarray(inputs["logit_scale"], dtype=np.float32).reshape(P, 1)

    nc = _get_nc()
    in_maps = make_in_maps(q, k, v, ls)
    res = run_bass_kernel_spmd(nc, in_maps, core_ids=list(range(NCORES)))
    out = np.concatenate(
        [r["out"].reshape(BLOC, P, D) for r in res.results], axis=0
    )
    return out


if __name__ == "__main__":
    rng = np.random.default_rng(0)
    inputs = {
        "q": rng.standard_normal((B, P, D), dtype=np.float32),
        "k": rng.standard_normal((B, N, D), dtype=np.float32),
        "v": rng.standard_normal((B, N, D), dtype=np.float32),
        "logit_scale": np.log(10.0 * np.ones((P, 1), dtype=np.float32)),
    }
    out = kernel(**inputs)
    print("out", out.shape, out.dtype)
